# revision 10
# baseline (speedup 1.0000x reference)
"""AttentiveItemToVec TRN2 kernel (8 NeuronCores, SPMD data-parallel over batch).

Math per batch row b (J=32 targets, M=100 contexts, E=128, DA=40):
  cos[j,m] = <tqn_j, ckn_m> with tqn/ckn the A-projected, per-VOCAB-normalized
             embeddings (norms are pure functions of the vocab row -> host).
  attn = softmax_m(cos + mask);  z = (attn @ u) @ W2^T + b2
             (W2 = R_w@Bc_w, b2 = R_w@Bc_b + R_b, using sum(attn)=1)

Device strategy per core (128 batch rows = 12800 c-tokens, 4096 t-tokens):
  - gather tables (host-precomputed, bf16):
      cfull [1M, 168] = [cvec | ckn],  tfull [1M, 41] = [tqn | 1.0]
  - 100 + 32 indirect row-gathers (128 rows each; the [P,1]-offset form is
    the only working indirect primitive, ~1.4us/instr on the gpsimd queue ->
    the wall-clock floor; everything else is hidden under the gather stream)
  - ALL transposes on the PE (tensor engine): XBAR DMA-transposes act as
    DMA-pipeline barriers and stall the gather descriptor stream ~5us each
  - pad mask folded into the dot matmul as contraction row 40 (lhsT row 40 =
    0/-1e30 per token, rhs row 40 = 1.0 from the table)
  - u m-major per-b layout via a DRAM bounce (direct DMAs; PE matmul
    operands must sit at base partition 0/32/64, so reading u straight out
    of the token-major gather tiles is not expressible)
  - endgame per 4-b group (128 tokens): z^T = W2 @ alphaT; PE-transpose to
    token-major, x 1/rowsum (per-partition scalar), + b2 (replicated const
    tile), store bf16
"""
import sys

sys.path.insert(0, "/opt/trn_rl_repo")

import numpy as np
import ml_dtypes

import concourse.bass as bass
import concourse.mybir as mybir
from concourse import bacc
from concourse.tile import TileContext
from concourse.bass_utils import run_bass_kernel_spmd

F32 = mybir.dt.float32
BF16 = mybir.dt.bfloat16
I32 = mybir.dt.int32
AF = mybir.ActivationFunctionType
OP = mybir.AluOpType

V, E, DA = 1_000_000, 128, 40
B, J, M = 1024, 32, 100
NCORES = 8
BL = B // NCORES            # 128 batch rows per core
CE = E + DA                 # 168 fused c row: [u(128) | ckn(40)]
TW = DA + 1                 # 41 t row: [tqn(40) | 1.0]
NTC = BL * M // 128         # 100 c-gather tiles
NTT = BL * J // 128         # 32 t-gather tiles
NTOK = BL * M               # 12800 c tokens
TTOK = BL * J               # 4096 t tokens
NCH = 4                     # chunks of 32 b's
TPC = NTC // NCH            # 25 c tiles per chunk
SUB = 5                     # c tiles per bounce sub-write
NEG = -1e30

_trace = [False]
_last_exec_ns = [None]


def _build_bass():
    nc = bacc.Bacc("TRN2", target_bir_lowering=False, debug=False,
                   num_devices=NCORES)

    cfull = nc.declare_dram_parameter("cfull", [V, CE], BF16, isOutput=False)
    tfull = nc.declare_dram_parameter("tfull", [V, TW], BF16, isOutput=False)
    cidx = nc.declare_dram_parameter("cidx", [128, NTC], I32, isOutput=False)
    tidx = nc.declare_dram_parameter("tidx", [128, NTT], I32, isOutput=False)
    negm = nc.declare_dram_parameter("negm", [1, NTOK], BF16, isOutput=False)
    w2t = nc.declare_dram_parameter("w2t", [E, E], BF16, isOutput=False)
    b2f = nc.declare_dram_parameter("b2f", [128, E], F32, isOutput=False)
    identd = nc.declare_dram_parameter("identd", [128, 128], BF16,
                                       isOutput=False)
    zout = nc.declare_dram_parameter("zout", [TTOK, E], BF16, isOutput=True)

    with TileContext(nc) as tc:
        with tc.tile_pool(name="const", bufs=1) as cp, \
             tc.tile_pool(name="big", bufs=1) as bigp, \
             tc.tile_pool(name="dram", bufs=1, space="DRAM") as dp, \
             tc.tile_pool(name="cg", bufs=8) as cgp, \
             tc.tile_pool(name="tg", bufs=4) as tgp, \
             tc.tile_pool(name="wrk", bufs=2) as wp, \
             tc.tile_pool(name="dotps", bufs=1, space="PSUM") as dotp, \
             tc.tile_pool(name="trps", bufs=1, space="PSUM") as trp, \
             tc.tile_pool(name="rs2ps", bufs=1, space="PSUM") as rs2p, \
             tc.tile_pool(name="zpsp", bufs=1, space="PSUM") as zpp, \
             tc.tile_pool(name="sqps", bufs=1, space="PSUM") as sqp:

            # ---- constants ----
            cidx_t = cp.tile([128, NTC], I32)
            nc.sync.dma_start(out=cidx_t[:], in_=cidx[:, :])
            tidx_t = cp.tile([128, NTT], I32)
            nc.sync.dma_start(out=tidx_t[:], in_=tidx[:, :])
            w2t_t = cp.tile([E, E], BF16)
            nc.sync.dma_start(out=w2t_t[:], in_=w2t[:, :])
            b2_t = cp.tile([128, E], F32)
            nc.sync.dma_start(out=b2_t[:], in_=b2f[:, :])
            ident = cp.tile([128, 128], BF16)
            nc.sync.dma_start(out=ident[:], in_=identd[:, :])
            ones100 = cp.tile([M, 1], BF16)
            nc.vector.memset(ones100[:], 1.0)

            # ---- persistent arrays ----
            cknT = bigp.tile([128, NTOK], BF16)      # rows 0:40 ckn, 40 mask
            tqnT = bigp.tile([128, TTOK], BF16)      # rows 0:40 tqn, 40 ones
            ET_all = bigp.tile([M, TTOK], BF16)      # exp(cos+mask), m-part
            alphaT = bigp.tile([E, TTOK], BF16)      # E-major alpha (unnorm)
            inv_all = bigp.tile([128, NTT], F32)     # 1/rowsum, token-major
            u_all = bigp.tile([M, BL, E], BF16)      # m-part, b, e

            ub_d = dp.tile([NTOK, E], BF16, name="ub_d")

            def emit_group(g):
                """4 b's: dot -> exp -> rowsum -> alpha -> endgame -> store."""
                nc.scalar.dma_start(
                    out=u_all[:, 4 * g:4 * (g + 1), :],
                    in_=ub_d[400 * g:400 * (g + 1), :]
                        .rearrange("(b m) e -> m b e", b=4))
                dps = dotp.tile([M, 128], F32, space="PSUM", tag="dot")
                for r in range(4):
                    b = 4 * g + r
                    nc.tensor.matmul(
                        dps[:, 32 * r:32 * (r + 1)],
                        cknT[0:TW, M * b:M * (b + 1)],
                        tqnT[0:TW, J * b:J * (b + 1)],
                        start=True, stop=True)
                sl = slice(128 * g, 128 * (g + 1))
                nc.scalar.activation(ET_all[:, sl], dps[:], AF.Exp)
                rs2 = rs2p.tile([128, 1], F32, space="PSUM", tag="rs2")
                nc.tensor.matmul(rs2[:], ET_all[:, sl], ones100[:],
                                 start=True, stop=True)
                nc.vector.reciprocal(inv_all[:, g:g + 1], rs2[:])
                aps = sqp.tile([E, 128], F32, space="PSUM", tag="sq", bufs=2)
                for r in range(4):
                    b = 4 * g + r
                    nc.tensor.matmul(
                        aps[:, 32 * r:32 * (r + 1)],
                        u_all[:, b, :], ET_all[:, J * b:J * (b + 1)],
                        start=True, stop=True)
                nc.vector.tensor_copy(alphaT[:, sl], aps[:])
                # endgame for these 128 tokens
                zps = zpp.tile([E, 128], F32, space="PSUM", tag="z")
                nc.tensor.matmul(zps[:], w2t_t[:], alphaT[:, sl],
                                 start=True, stop=True)
                zfin = wp.tile([E, 128], BF16, tag="zfin")
                nc.vector.tensor_copy(zfin[:], zps[:])
                ztp = sqp.tile([128, 128], BF16, space="PSUM", tag="sqz",
                               bufs=1, name=f"ztp_{g}")
                nc.tensor.transpose(ztp[:], zfin[:], ident[:])
                zmul = wp.tile([128, E], F32, tag="zmul")
                nc.vector.tensor_scalar_mul(zmul[:], ztp[:],
                                            inv_all[:, g:g + 1])
                ztk = wp.tile([128, E], BF16, tag="ztk")
                nc.vector.tensor_tensor(out=ztk[:], in0=zmul[:],
                                        in1=b2_t[:], op=OP.add)
                nc.scalar.dma_start(out=zout[128 * g:128 * (g + 1), :],
                                    in_=ztk[:])

            for q in range(NCH):
                # mask row for this chunk (row 40; disjoint from ck copies)
                c0 = NTOK // NCH * q
                nc.scalar.dma_start(
                    out=cknT[DA:DA + 1, c0:c0 + NTOK // NCH],
                    in_=negm[:, c0:c0 + NTOK // NCH])
                # -- t gathers + PE transposes (batched into one PSUM tile) --
                for th in range(2):
                    t_tile = tgp.tile([128, 4, TW], BF16, tag="tg")
                    for i in range(4):
                        st = 8 * q + 4 * th + i
                        nc.gpsimd.indirect_dma_start(
                            out=t_tile[:, i, :], out_offset=None,
                            in_=tfull[:, :],
                            in_offset=bass.IndirectOffsetOnAxis(
                                ap=tidx_t[:, st:st + 1], axis=0))
                    tqp = trp.tile([TW, 128 * SUB], BF16, space="PSUM",
                                   tag="tr")
                    for i in range(4):
                        nc.tensor.transpose(tqp[:, 128 * i:128 * (i + 1)],
                                            t_tile[:, i, :], ident[:])
                    t0 = 128 * (8 * q + 4 * th)
                    nc.vector.tensor_copy(tqnT[0:TW, t0:t0 + 512],
                                          tqp[:, 0:512])
                # -- c gathers; batched ck transposes; compute per group --
                glast = 8 * q  # next group to emit
                for sgrp in range(TPC // SUB):
                    c_tile = cgp.tile([128, SUB, CE], BF16, tag="cg")
                    for i in range(SUB):
                        jt = TPC * q + SUB * sgrp + i
                        nc.gpsimd.indirect_dma_start(
                            out=c_tile[:, i, :], out_offset=None,
                            in_=cfull[:, :],
                            in_offset=bass.IndirectOffsetOnAxis(
                                ap=cidx_t[:, jt:jt + 1], axis=0))
                    r0 = 128 * (TPC * q + SUB * sgrp)
                    nc.sync.dma_start(
                        out=ub_d[r0:r0 + 128 * SUB, :].rearrange(
                            "(i p) e -> p i e", p=128),
                        in_=c_tile[:, :, 0:E])
                    ckp = trp.tile([TW, 128 * SUB], BF16, space="PSUM",
                                   tag="tr")
                    for i in range(SUB):
                        nc.tensor.transpose(ckp[0:DA, 128 * i:128 * (i + 1)],
                                            c_tile[:, i, E:CE], ident[:])
                    nc.vector.tensor_copy(
                        cknT[0:DA, r0:r0 + 128 * SUB], ckp[0:DA, :])
                    # groups fully covered by tokens gathered so far
                    tok_done = 640 * (sgrp + 1)
                    while glast < 8 * (q + 1) and \
                            400 * (glast - 8 * q + 1) <= tok_done:
                        emit_group(glast)
                        glast += 1
                while glast < 8 * (q + 1):
                    emit_group(glast)
                    glast += 1

    nc.finalize()
    return nc


_nc_cache = [None]


def kernel(batch_titems, batch_citems, pad_rows, pad_cols, tvec, cvec,
           Ac_w, Ac_b, At_w, At_b, Bc_w, Bc_b, R_w, R_b):
    batch_titems = np.asarray(batch_titems).astype(np.int32)
    batch_citems = np.asarray(batch_citems).astype(np.int32)
    pad_rows = np.asarray(pad_rows).astype(np.int64)
    pad_cols = np.asarray(pad_cols).astype(np.int64)
    tvec = np.asarray(tvec, dtype=np.float32)
    cvec = np.asarray(cvec, dtype=np.float32)
    Ac_w = np.asarray(Ac_w, dtype=np.float32)
    Ac_b = np.asarray(Ac_b, dtype=np.float32)
    At_w = np.asarray(At_w, dtype=np.float32)
    At_b = np.asarray(At_b, dtype=np.float32)
    Bc_w = np.asarray(Bc_w, dtype=np.float32)
    Bc_b = np.asarray(Bc_b, dtype=np.float32)
    R_w = np.asarray(R_w, dtype=np.float32)
    R_b = np.asarray(R_b, dtype=np.float32)

    # ---- host folding: normalized projection tables, fused W2/b2 ----
    ck = cvec @ Ac_w.T + Ac_b                        # [V, 40]
    nck = np.maximum(np.linalg.norm(ck, axis=1, keepdims=True), 1e-6)
    cfull = np.empty((V, CE), dtype=ml_dtypes.bfloat16)
    cfull[:, :E] = cvec.astype(ml_dtypes.bfloat16)
    cfull[:, E:] = (ck / nck).astype(ml_dtypes.bfloat16)
    tq = tvec @ At_w.T + At_b                        # [V, 40]
    ntq = np.maximum(np.linalg.norm(tq, axis=1, keepdims=True), 1e-6)
    tfull = np.ones((V, TW), dtype=ml_dtypes.bfloat16)
    tfull[:, :DA] = (tq / ntq).astype(ml_dtypes.bfloat16)
    W2 = R_w @ Bc_w
    w2t = np.ascontiguousarray(W2.T).astype(ml_dtypes.bfloat16)
    b2 = (R_w @ Bc_b + R_b).astype(np.float32)
    b2f = np.ascontiguousarray(np.broadcast_to(b2, (128, E)))
    identd = np.eye(128, dtype=np.float32).astype(ml_dtypes.bfloat16)

    in_maps = []
    for c in range(NCORES):
        b0 = c * BL
        cit = batch_citems[b0:b0 + BL].ravel()       # [12800] b-major
        tit = batch_titems[b0:b0 + BL].ravel()       # [4096]
        cidx = np.ascontiguousarray(cit.reshape(NTC, 128).T.astype(np.int32))
        tidx = np.ascontiguousarray(tit.reshape(NTT, 128).T.astype(np.int32))
        sel = (pad_rows >= b0) & (pad_rows < b0 + BL)
        negm = np.zeros((1, NTOK), dtype=ml_dtypes.bfloat16)
        flat = (pad_rows[sel] - b0) * M + pad_cols[sel]
        negm[0, flat] = NEG
        in_maps.append({
            "cfull": cfull, "tfull": tfull,
            "cidx": cidx, "tidx": tidx, "negm": negm,
            "w2t": w2t, "b2f": b2f, "identd": identd,
        })

    if _nc_cache[0] is None:
        _nc_cache[0] = _build_bass()
    nc = _nc_cache[0]

    res = run_bass_kernel_spmd(nc, in_maps, list(range(NCORES)),
                               trace=_trace[0])
    _last_exec_ns[0] = res.exec_time_ns
    z = np.stack([r["zout"].astype(np.float32).reshape(BL, J, E)
                  for r in res.results], axis=0)
    return z.reshape(B, J, E)


# revision 11
# speedup vs baseline: 1.1838x; 1.1838x over previous
"""AttentiveItemToVec TRN2 kernel (8 NeuronCores, SPMD data-parallel over batch).

Math per batch row b (J=32 targets, M=100 contexts, E=128, DA=40):
  cos[j,m] = <tqn_j, ckn_m> with tqn/ckn the A-projected, per-VOCAB-normalized
             embeddings (norms are pure functions of the vocab row -> host).
  attn = softmax_m(cos + mask);  z = (attn @ u) @ W2^T + b2
             (W2 = R_w@Bc_w, b2 = R_w@Bc_b + R_b, using sum(attn)=1)

Device strategy per core (128 batch rows = 12800 c-tokens, 4096 t-tokens):
  - gather tables (host-precomputed, bf16):
      cfull [1M, 168] = [cvec | ckn],  tfull [1M, 41] = [tqn | 1.0]
  - 100 + 32 indirect row-gathers (128 rows each; the [P,1]-offset form is
    the only working indirect primitive, ~1.4us/instr on the gpsimd queue ->
    the wall-clock floor; everything else is hidden under the gather stream)
  - ALL transposes on the PE (tensor engine): XBAR DMA-transposes act as
    DMA-pipeline barriers and stall the gather descriptor stream ~5us each
  - pad mask folded into the dot matmul as contraction row 40 (lhsT row 40 =
    0/-1e30 per token, rhs row 40 = 1.0 from the table)
  - u m-major per-b layout via a DRAM bounce (direct DMAs; PE matmul
    operands must sit at base partition 0/32/64, so reading u straight out
    of the token-major gather tiles is not expressible)
  - endgame per 4-b group (128 tokens): z^T = W2 @ alphaT; PE-transpose to
    token-major, x 1/rowsum (per-partition scalar), + b2 (replicated const
    tile), store bf16
"""
import sys

sys.path.insert(0, "/opt/trn_rl_repo")

import numpy as np
import ml_dtypes

import concourse.bass as bass
import concourse.mybir as mybir
from concourse import bacc
from concourse.tile import TileContext
from concourse.bass_utils import run_bass_kernel_spmd

F32 = mybir.dt.float32
BF16 = mybir.dt.bfloat16
I32 = mybir.dt.int32
AF = mybir.ActivationFunctionType
OP = mybir.AluOpType

V, E, DA = 1_000_000, 128, 40
B, J, M = 1024, 32, 100
NCORES = 8
BL = B // NCORES            # 128 batch rows per core
CE = E + DA                 # 168 fused c row: [u(128) | ckn(40)]
TW = DA + 1                 # 41 t row: [tqn(40) | 1.0]
NTC = BL * M // 128         # 100 c-gather tiles
NTT = BL * J // 128         # 32 t-gather tiles
NTOK = BL * M               # 12800 c tokens
TTOK = BL * J               # 4096 t tokens
NCH = 4                     # chunks of 32 b's
TPC = NTC // NCH            # 25 c tiles per chunk
SUB = 5                     # c tiles per bounce sub-write
NEG = -1e30

_trace = [False]
_last_exec_ns = [None]


def _build_bass():
    nc = bacc.Bacc("TRN2", target_bir_lowering=False, debug=False,
                   num_devices=NCORES)

    cfull = nc.declare_dram_parameter("cfull", [V, CE], BF16, isOutput=False)
    tfull = nc.declare_dram_parameter("tfull", [V, TW], BF16, isOutput=False)
    cidx = nc.declare_dram_parameter("cidx", [128, NTC], I32, isOutput=False)
    tidx = nc.declare_dram_parameter("tidx", [128, NTT], I32, isOutput=False)
    negm = nc.declare_dram_parameter("negm", [1, NTOK], BF16, isOutput=False)
    w2t = nc.declare_dram_parameter("w2t", [E, E], BF16, isOutput=False)
    b2f = nc.declare_dram_parameter("b2f", [128, E], F32, isOutput=False)
    identd = nc.declare_dram_parameter("identd", [128, 128], BF16,
                                       isOutput=False)
    zout = nc.declare_dram_parameter("zout", [TTOK, E], BF16, isOutput=True)

    with TileContext(nc) as tc:
        with tc.tile_pool(name="const", bufs=1) as cp, \
             tc.tile_pool(name="big", bufs=1) as bigp, \
             tc.tile_pool(name="dram", bufs=1, space="DRAM") as dp, \
             tc.tile_pool(name="cg", bufs=8) as cgp, \
             tc.tile_pool(name="tg", bufs=4) as tgp, \
             tc.tile_pool(name="wrk", bufs=2) as wp, \
             tc.tile_pool(name="dotps", bufs=1, space="PSUM") as dotp, \
             tc.tile_pool(name="trps", bufs=1, space="PSUM") as trp, \
             tc.tile_pool(name="rs2ps", bufs=1, space="PSUM") as rs2p, \
             tc.tile_pool(name="zpsp", bufs=1, space="PSUM") as zpp, \
             tc.tile_pool(name="sqps", bufs=1, space="PSUM") as sqp:

            # ---- constants ----
            cidx_t = cp.tile([128, NTC], I32)
            nc.sync.dma_start(out=cidx_t[:], in_=cidx[:, :])
            tidx_t = cp.tile([128, NTT], I32)
            nc.sync.dma_start(out=tidx_t[:], in_=tidx[:, :])
            w2t_t = cp.tile([E, E], BF16)
            nc.sync.dma_start(out=w2t_t[:], in_=w2t[:, :])
            b2_t = cp.tile([128, E], F32)
            nc.sync.dma_start(out=b2_t[:], in_=b2f[:, :])
            ident = cp.tile([128, 128], BF16)
            nc.sync.dma_start(out=ident[:], in_=identd[:, :])
            ones100 = cp.tile([M, 1], BF16)
            nc.vector.memset(ones100[:], 1.0)

            # ---- persistent arrays ----
            cknT = bigp.tile([128, NTOK], BF16)      # rows 0:40 ckn, 40 mask
            tqnT = bigp.tile([128, TTOK], BF16)      # rows 0:40 tqn, 40 ones
            ET_all = bigp.tile([M, TTOK], BF16)      # exp(cos+mask), m-part
            alphaT = bigp.tile([E, TTOK], BF16)      # E-major alpha (unnorm)
            inv_all = bigp.tile([128, NTT], F32)     # 1/rowsum, token-major
            u_all = bigp.tile([M, BL, E], BF16)      # m-part, b, e

            ub_d = dp.tile([NTOK, E], BF16, name="ub_d")

            def emit_group(g):
                """4 b's: dot -> exp -> rowsum -> alpha -> endgame -> store."""
                nc.sync.dma_start(
                    out=u_all[:, 4 * g:4 * (g + 1), :],
                    in_=ub_d[400 * g:400 * (g + 1), :]
                        .rearrange("(b m) e -> m b e", b=4))
                dps = dotp.tile([M, 128], F32, space="PSUM", tag="dot")
                for r in range(4):
                    b = 4 * g + r
                    nc.tensor.matmul(
                        dps[:, 32 * r:32 * (r + 1)],
                        cknT[0:TW, M * b:M * (b + 1)],
                        tqnT[0:TW, J * b:J * (b + 1)],
                        start=True, stop=True)
                sl = slice(128 * g, 128 * (g + 1))
                nc.scalar.activation(ET_all[:, sl], dps[:], AF.Exp)
                rs2 = rs2p.tile([128, 1], F32, space="PSUM", tag="rs2")
                nc.tensor.matmul(rs2[:], ET_all[:, sl], ones100[:],
                                 start=True, stop=True)
                nc.vector.reciprocal(inv_all[:, g:g + 1], rs2[:])
                aps = sqp.tile([E, 128], F32, space="PSUM", tag="sq", bufs=2)
                for r in range(4):
                    b = 4 * g + r
                    nc.tensor.matmul(
                        aps[:, 32 * r:32 * (r + 1)],
                        u_all[:, b, :], ET_all[:, J * b:J * (b + 1)],
                        start=True, stop=True)
                nc.vector.tensor_copy(alphaT[:, sl], aps[:])
                # endgame for these 128 tokens
                zps = zpp.tile([E, 128], F32, space="PSUM", tag="z")
                nc.tensor.matmul(zps[:], w2t_t[:], alphaT[:, sl],
                                 start=True, stop=True)
                zfin = wp.tile([E, 128], BF16, tag="zfin")
                nc.vector.tensor_copy(zfin[:], zps[:])
                ztp = sqp.tile([128, 128], BF16, space="PSUM", tag="sqz",
                               bufs=1, name=f"ztp_{g}")
                nc.tensor.transpose(ztp[:], zfin[:], ident[:])
                zmul = wp.tile([128, E], F32, tag="zmul")
                nc.vector.tensor_scalar_mul(zmul[:], ztp[:],
                                            inv_all[:, g:g + 1])
                ztk = wp.tile([128, E], BF16, tag="ztk")
                nc.vector.tensor_tensor(out=ztk[:], in0=zmul[:],
                                        in1=b2_t[:], op=OP.add)
                nc.sync.dma_start(out=zout[128 * g:128 * (g + 1), :],
                                  in_=ztk[:])

            for q in range(NCH):
                # mask row for this chunk (row 40; disjoint from ck copies)
                c0 = NTOK // NCH * q
                nc.scalar.dma_start(
                    out=cknT[DA:DA + 1, c0:c0 + NTOK // NCH],
                    in_=negm[:, c0:c0 + NTOK // NCH])
                # -- t gathers + PE transposes (batched into one PSUM tile) --
                for th in range(2):
                    t_tile = tgp.tile([128, 4, TW], BF16, tag="tg")
                    for i in range(4):
                        st = 8 * q + 4 * th + i
                        nc.gpsimd.indirect_dma_start(
                            out=t_tile[:, i, :], out_offset=None,
                            in_=tfull[:, :],
                            in_offset=bass.IndirectOffsetOnAxis(
                                ap=tidx_t[:, st:st + 1], axis=0))
                    tqp = trp.tile([TW, 128 * SUB], BF16, space="PSUM",
                                   tag="tr")
                    for i in range(4):
                        nc.tensor.transpose(tqp[:, 128 * i:128 * (i + 1)],
                                            t_tile[:, i, :], ident[:])
                    t0 = 128 * (8 * q + 4 * th)
                    nc.vector.tensor_copy(tqnT[0:TW, t0:t0 + 512],
                                          tqp[:, 0:512])
                # -- c gathers; batched ck transposes; compute per group --
                glast = 8 * q  # next group to emit
                for sgrp in range(TPC // SUB):
                    c_tile = cgp.tile([128, SUB, CE], BF16, tag="cg")
                    for i in range(SUB):
                        jt = TPC * q + SUB * sgrp + i
                        nc.gpsimd.indirect_dma_start(
                            out=c_tile[:, i, :], out_offset=None,
                            in_=cfull[:, :],
                            in_offset=bass.IndirectOffsetOnAxis(
                                ap=cidx_t[:, jt:jt + 1], axis=0))
                    r0 = 128 * (TPC * q + SUB * sgrp)
                    nc.sync.dma_start(
                        out=ub_d[r0:r0 + 128 * SUB, :].rearrange(
                            "(i p) e -> p i e", p=128),
                        in_=c_tile[:, :, 0:E])
                    ckp = trp.tile([TW, 128 * SUB], BF16, space="PSUM",
                                   tag="tr")
                    for i in range(SUB):
                        nc.tensor.transpose(ckp[0:DA, 128 * i:128 * (i + 1)],
                                            c_tile[:, i, E:CE], ident[:])
                    nc.vector.tensor_copy(
                        cknT[0:DA, r0:r0 + 128 * SUB], ckp[0:DA, :])
                    # groups fully covered by tokens gathered so far
                    tok_done = 640 * (sgrp + 1)
                    while glast < 8 * (q + 1) and \
                            400 * (glast - 8 * q + 1) <= tok_done:
                        emit_group(glast)
                        glast += 1
                while glast < 8 * (q + 1):
                    emit_group(glast)
                    glast += 1

    nc.finalize()
    return nc


_nc_cache = [None]


def kernel(batch_titems, batch_citems, pad_rows, pad_cols, tvec, cvec,
           Ac_w, Ac_b, At_w, At_b, Bc_w, Bc_b, R_w, R_b):
    batch_titems = np.asarray(batch_titems).astype(np.int32)
    batch_citems = np.asarray(batch_citems).astype(np.int32)
    pad_rows = np.asarray(pad_rows).astype(np.int64)
    pad_cols = np.asarray(pad_cols).astype(np.int64)
    tvec = np.asarray(tvec, dtype=np.float32)
    cvec = np.asarray(cvec, dtype=np.float32)
    Ac_w = np.asarray(Ac_w, dtype=np.float32)
    Ac_b = np.asarray(Ac_b, dtype=np.float32)
    At_w = np.asarray(At_w, dtype=np.float32)
    At_b = np.asarray(At_b, dtype=np.float32)
    Bc_w = np.asarray(Bc_w, dtype=np.float32)
    Bc_b = np.asarray(Bc_b, dtype=np.float32)
    R_w = np.asarray(R_w, dtype=np.float32)
    R_b = np.asarray(R_b, dtype=np.float32)

    # ---- host folding: normalized projection tables, fused W2/b2 ----
    ck = cvec @ Ac_w.T + Ac_b                        # [V, 40]
    nck = np.maximum(np.linalg.norm(ck, axis=1, keepdims=True), 1e-6)
    cfull = np.empty((V, CE), dtype=ml_dtypes.bfloat16)
    cfull[:, :E] = cvec.astype(ml_dtypes.bfloat16)
    cfull[:, E:] = (ck / nck).astype(ml_dtypes.bfloat16)
    tq = tvec @ At_w.T + At_b                        # [V, 40]
    ntq = np.maximum(np.linalg.norm(tq, axis=1, keepdims=True), 1e-6)
    tfull = np.ones((V, TW), dtype=ml_dtypes.bfloat16)
    tfull[:, :DA] = (tq / ntq).astype(ml_dtypes.bfloat16)
    W2 = R_w @ Bc_w
    w2t = np.ascontiguousarray(W2.T).astype(ml_dtypes.bfloat16)
    b2 = (R_w @ Bc_b + R_b).astype(np.float32)
    b2f = np.ascontiguousarray(np.broadcast_to(b2, (128, E)))
    identd = np.eye(128, dtype=np.float32).astype(ml_dtypes.bfloat16)

    in_maps = []
    for c in range(NCORES):
        b0 = c * BL
        cit = batch_citems[b0:b0 + BL].ravel()       # [12800] b-major
        tit = batch_titems[b0:b0 + BL].ravel()       # [4096]
        cidx = np.ascontiguousarray(cit.reshape(NTC, 128).T.astype(np.int32))
        tidx = np.ascontiguousarray(tit.reshape(NTT, 128).T.astype(np.int32))
        sel = (pad_rows >= b0) & (pad_rows < b0 + BL)
        negm = np.zeros((1, NTOK), dtype=ml_dtypes.bfloat16)
        flat = (pad_rows[sel] - b0) * M + pad_cols[sel]
        negm[0, flat] = NEG
        in_maps.append({
            "cfull": cfull, "tfull": tfull,
            "cidx": cidx, "tidx": tidx, "negm": negm,
            "w2t": w2t, "b2f": b2f, "identd": identd,
        })

    if _nc_cache[0] is None:
        _nc_cache[0] = _build_bass()
    nc = _nc_cache[0]

    res = run_bass_kernel_spmd(nc, in_maps, list(range(NCORES)),
                               trace=_trace[0])
    _last_exec_ns[0] = res.exec_time_ns
    z = np.stack([r["zout"].astype(np.float32).reshape(BL, J, E)
                  for r in res.results], axis=0)
    return z.reshape(B, J, E)
